# revision 60
# baseline (speedup 1.0000x reference)
"""Causal self-attention (B=2, T=2048, C=1024, H=16) on 8 trn2 NeuronCores.

Sharding: batch x head-group. Core c handles batch c//4 and the 4 heads
[4*(c%4), 4*(c%4)+4), as two head-pairs A=(h0,h1), B=(h2,h3). Each core
reads only its batch's half of x (4MB bf16) and writes an 8MB partial
output; the host sums 4 partials per batch and adds b_proj.

Per core:
  - QKV projection of its batch (6 groups of 128 weight cols: qA kA vA
    qB kB vB), producing qT/kT in [head_dim, T] layout and V' in
    [T, head_dim] layout via PE transposes, with a ones column per head
    (softmax denominator accumulates during att@V).
  - Flash-style causal attention per (head-pair, q-block) as ONE
    continuous software-pipelined stream across all 8 blocks in
    pair-interleaved order (A,0),(B,0),(A,1),(B,1),...: concurrent
    row-group S^T matmul pairs into a 2-bank PSUM tile, one ACT exp for
    both heads, triangular 0/1 mask on diagonal tiles (GpSimd), att@V
    pipelined two k-tiles behind S/exp with the att@V pend queue
    surviving block boundaries (no pipeline drain between blocks).
  - A FIFO feed queue interleaves one filler work item per k-tile into
    the PE stream: remaining QKV group chains, x-tile DMAs, softmax
    normalizations (selector-matmul denominator broadcast + fast approx
    reciprocal + in-place multiply) and output-projection chunks. When
    the queue is empty a tiny junk matmul keeps the PE HAM clock gate
    at 8/8 through ACT(exp)-bound stretches.
  - Output projection contracts 256 y-dims (both head-pairs) per
    128-query chunk; partial [2048, 1024] written to DRAM.

Matmuls run in bfloat16 (same PE column throughput as f32r, but half
the DMA/SBUF traffic and lower power -> less HAM/P0 throttling), with
full fp32 PSUM accumulate. rel err ~3e-3 vs the 2e-2 gate.
"""

import sys

sys.path.insert(0, "/opt/trn_rl_repo")

import numpy as np

B, T, C, H, HD = 2, 2048, 1024, 16, 64
NCORE = 8
HPC = 4           # heads per core
NT = T // 512     # 4 T-tiles (one batch per core)
CCH = C // 128    # 8 contraction chunks
NKT = T // 128    # 16 k-tiles

JUNK_N = 160      # keepalive junk matmul width (columns)


def _to_bf16(x):
    import ml_dtypes

    return np.ascontiguousarray(np.asarray(x, dtype=np.float32)).astype(
        ml_dtypes.bfloat16
    )


_CACHE = {}


def _build():
    if "nc" in _CACHE:
        return _CACHE["nc"]
    from contextlib import ExitStack

    import concourse.bass as bass
    import concourse.bacc as bacc
    import concourse.mybir as mybir
    import concourse.tile as tile
    from concourse.masks import make_identity, make_upper_triangular

    f32, bf16 = mybir.dt.float32, mybir.dt.bfloat16
    AF = mybir.ActivationFunctionType

    nc = bacc.Bacc(None, target_bir_lowering=False, debug=False)
    # x pre-permuted on host to [p, tt, cc, t] so each T-tile DMA reads
    # contiguous runs per partition
    xT_d = nc.dram_tensor("xT", [128, NT, CCH, 512], bf16, kind="ExternalInput")
    # weights group-major so the first QKV chain can start after 1 group
    wqkv_d = nc.dram_tensor("wqkv", [128, 6, CCH, 128], bf16, kind="ExternalInput")
    bqkv_d = nc.dram_tensor("bqkv", [128, 6], f32, kind="ExternalInput")
    wp_d = nc.dram_tensor("wp", [128, 2, C], bf16, kind="ExternalInput")
    sel_d = nc.dram_tensor("sel", [33, 128], bf16, kind="ExternalInput")
    # bf16 partials: host sums the 4 per-batch partials in float64
    out_d = nc.dram_tensor("out", [T, C], bf16, kind="ExternalOutput")

    with tile.TileContext(nc) as tc, ExitStack() as ctx:
        sb = ctx.enter_context(tc.tile_pool(name="sb", bufs=1))
        xp = ctx.enter_context(tc.tile_pool(name="xp", bufs=4))
        esp = ctx.enter_context(tc.tile_pool(name="esp", bufs=4))
        rcpp = ctx.enter_context(tc.tile_pool(name="rcpp", bufs=2))
        outp = ctx.enter_context(tc.tile_pool(name="outp", bufs=3))
        # PSUM: 2 + 4 + 2 = 8 banks
        pa = ctx.enter_context(tc.tile_pool(name="pa", bufs=2, space="PSUM"))
        pss = ctx.enter_context(tc.tile_pool(name="pss", bufs=2, space="PSUM"))
        pso = ctx.enter_context(tc.tile_pool(name="pso", bufs=2, space="PSUM"))

        wq_sb = sb.tile([128, 6, CCH, 128], bf16, tag="wq")

        qT = [sb.tile([128, T], bf16, tag=f"qT{hp}", name=f"qT{hp}") for hp in range(2)]
        kT = [sb.tile([128, T], bf16, tag=f"kT{hp}", name=f"kT{hp}") for hp in range(2)]
        # V' per head-pair: per k-tile [V_h0 (64) | 1 | V_h1 (64) | 1] = 130;
        # the ones column accumulates the softmax denominator during att@v
        VW = 130
        Vp = [
            sb.tile([128, NKT, VW], bf16, tag=f"Vp{hp}", name=f"Vp{hp}")
            for hp in range(2)
        ]
        for hp in range(2):
            nc.vector.memset(Vp[hp][:, :, :], 1.0)
        # normalized attention outputs, persistent across all blocks
        ytb = [
            [
                sb.tile([128, 512], bf16, tag=f"ytb{hp}{qb}", name=f"ytb{hp}{qb}")
                for qb in range(NT)
            ]
            for hp in range(2)
        ]
        # selector [33, 128]: col j reads partition 0 (head0 denom) for j<64,
        # partition 32 (head1 denom) for j>=64 -- one K=33 matmul broadcasts
        # both heads' denominator rows into a single [128, 512] PSUM bank
        # (32-aligned partition bases are an ISA requirement)
        sel_sb = sb.tile([33, 128], bf16, tag="sel")
        nc.sync.dma_start(out=sel_sb, in_=sel_d[:, :])
        # one persistent denominator tile per block (norms run lazily many
        # slots after evacuation); rows 1..31/33.. are never written and must
        # be finite (the selector matmul touches all 33 partitions), so
        # memset the whole tiles once
        dh_tiles = {
            (hp, qb): sb.tile([33, 512], bf16, tag=f"dh{hp}{qb}", name=f"dh{hp}{qb}")
            for hp in range(2)
            for qb in range(NT)
        }
        for t in dh_tiles.values():
            nc.vector.memset(t[:, :], 1.0)

        # ---------------- QKV projection ----------------
        x_tiles = [None] * NT

        def emit_x_dma(tt):
            x_t = xp.tile([128, CCH, 512], bf16, tag="x", name=f"x{tt}")
            if tt == 0:
                # chunked loads so the first matmul chain starts early
                nc.sync.dma_start(out=wq_sb[:, 0], in_=wqkv_d[:, 0])
                for cc in range(CCH):
                    nc.sync.dma_start(out=x_t[:, cc, :], in_=xT_d[:, 0, cc, :])
                for g in range(1, 6):
                    nc.sync.dma_start(out=wq_sb[:, g], in_=wqkv_d[:, g])
            else:
                nc.sync.dma_start(out=x_t, in_=xT_d[:, tt, :, :])
            x_tiles[tt] = x_t

        def emit_qkv_group(tt, g):
            # g: 0=qA 1=kA 2=vA 3=qB 4=kB 5=vB
            hp, kind = divmod(g, 3)
            x_t = x_tiles[tt]
            tloc = tt * 512
            if kind == 2:
                ps = pa.tile([128, 4, 128], f32, tag="mm")
                for j in range(4):
                    for cc in range(CCH):
                        nc.tensor.matmul(
                            ps[:, j, :],
                            x_t[:, cc, j * 128 : (j + 1) * 128],
                            wq_sb[:, g, cc, :],
                            start=(cc == 0),
                            stop=(cc == CCH - 1),
                        )
            else:
                ps = pa.tile([128, 512], f32, tag="mm")
                for cc in range(CCH):
                    nc.tensor.matmul(
                        ps,
                        wq_sb[:, g, cc, :],
                        x_t[:, cc, :],
                        start=(cc == 0),
                        stop=(cc == CCH - 1),
                    )
            if kind == 0:
                nc.vector.tensor_scalar_add(
                    qT[hp][:, tloc : tloc + 512], ps, bias_sb[:, g : g + 1]
                )
            elif kind == 1:
                nc.vector.tensor_scalar_add(
                    kT[hp][:, tloc : tloc + 512], ps, bias_sb[:, g : g + 1]
                )
            else:
                # v is produced directly in [k-pos, v-dim] layout by swapping
                # matmul operands (x chunk stationary, w moving): no PE
                # transposes, no v_t staging on DVE. The v bias is folded
                # into b_proj on the host (softmax rows sum to 1).
                for j in range(4):
                    ktl = tt * 4 + j
                    # one strided copy moves both heads' V columns
                    nc.vector.tensor_copy(
                        Vp[hp][:, ktl, 0:130].rearrange("p (s e) -> p s e", s=2)[
                            :, :, 0:64
                        ],
                        ps[:, j, :].rearrange("p (s e) -> p s e", s=2),
                    )

        emit_x_dma(0)
        # constants not needed immediately: emit loads after the x chunks
        bias_sb = sb.tile([128, 6], f32, tag="bias")
        nc.sync.dma_start(out=bias_sb, in_=bqkv_d[:, :])
        emit_x_dma(1)
        wp_sb = sb.tile([128, 2, C], bf16, tag="wp")
        nc.sync.dma_start(out=wp_sb, in_=wp_d[:, :, :])
        ident = sb.tile([128, 128], bf16, tag="ident")
        make_identity(nc, ident)
        tri2 = sb.tile([128, 2, 128], bf16, tag="tri2")
        make_upper_triangular(nc, tri2[:, 0, :], val=1.0, diag=True)
        nc.gpsimd.tensor_copy(tri2[:, 1, :], tri2[:, 0, :])
        # PE warm-up: junk matmuls during the initial DMA wait pull the HAM
        # clock gate to 8/8 before the first real qkv chain issues. The
        # activity window is 4096 cycles (~3.4us) and must be filled
        # continuously, so emit ~4.5us worth of cold-rate matmuls.
        wu = pa.tile([128, 128], f32, tag="mm", name="warmup")
        for _ in range(42):
            nc.tensor.matmul(wu, ident, ident, start=True, stop=True)
        # preload the exp activation table set (~2.7us) off the critical path
        etp = sb.tile([1, 16], f32, tag="etp")
        nc.vector.memset(etp[:, :], 0.0)
        nc.scalar.activation(etp, etp, AF.Exp)

        # ------------- attention: one continuous pipelined stream -------------
        scale = 1.0 / 8.0  # 1/sqrt(HD)
        feeds = []        # FIFO of filler closures, one consumed per k-tile
        pend = []         # att@v groups pipelined two k-tiles behind S/exp

        def issue_pend():
            grp, on_done = pend.pop(0)
            for mm in grp:
                nc.tensor.matmul(**mm)
            if on_done is not None:
                on_done()

        def emit_keepalive():
            # throwaway matmul into a fresh pa tile: keeps the PE HAM
            # activity window busy through ACT(exp)-bound stretches so the
            # real att matmuls stay at the 2.4GHz clock
            junk = pa.tile([128, JUNK_N], f32, tag="mm", name="junk")
            nc.tensor.matmul(
                junk, ident[0:64, :], qT[0][0:64, 0:JUNK_N], start=True, stop=True
            )

        def make_norm(dh, hp, qb):
            def norm():
                # broadcast both heads' denominators into one PSUM bank via a
                # single K=33 matmul (selector stationary), one fast approx
                # reciprocal, then normalize ytb in place.
                den = pa.tile([128, 512], f32, tag="mm", name="den")
                nc.tensor.matmul(
                    den, sel_sb[0:33, :], dh[0:33, :], start=True, stop=True
                )
                rcp = rcpp.tile([128, 512], f32, tag="rcp")
                nc.vector.reciprocal_approx_fast(out=rcp, in_=den)
                yt = ytb[hp][qb]
                nc.vector.tensor_mul(yt[:, :], yt[:, :], rcp[:, :])
            return norm

        def make_proj(qb, j):
            def proj():
                out_t = outp.tile([128, C], bf16, tag="out", name="out_t")
                js = slice(j * 128, (j + 1) * 128)
                for ncol in range(2):
                    cs = slice(ncol * 512, (ncol + 1) * 512)
                    pp = pa.tile([128, 512], f32, tag="mm", name="pp")
                    nc.tensor.matmul(
                        pp, ytb[0][qb][:, js], wp_sb[:, 0, cs], start=True, stop=False
                    )
                    nc.tensor.matmul(
                        pp, ytb[1][qb][:, js], wp_sb[:, 1, cs], start=False, stop=True
                    )
                    nc.vector.tensor_copy(out_t[:, cs], pp)
                row = qb * 512 + j * 128
                nc.sync.dma_start(out=out_d[row : row + 128, :], in_=out_t)
            return proj

        def make_evac(po, hp, qb):
            def evac():
                # evacuate po fast so the next block's att@v gets its PSUM
                # banks: denominator rows -> dh partitions 0/32, un-normalized
                # y -> ytb; then queue the (lazy) normalization and, once both
                # head-pairs of qb are evacuated, the output projection.
                dh = dh_tiles[(hp, qb)]
                yt = ytb[hp][qb]
                nc.vector.tensor_copy(dh[0:1, :], po[0][64:65, :])
                nc.vector.tensor_copy(yt[0:64, :], po[0][0:64, :])
                nc.vector.tensor_copy(dh[32:33, :], po[1][64:65, :])
                nc.vector.tensor_copy(yt[64:128, :], po[1][0:64, :])
                feeds.append(make_norm(dh, hp, qb))
                if hp == 1:
                    for j in range(4):
                        feeds.append(make_proj(qb, j))
            return evac

        def emit_att_block(hp, qb):
            n_kt = 4 * (qb + 1)
            po = [
                pso.tile([128, 512], f32, tag="po", name=f"po{hp}{qb}{h}")
                for h in range(2)
            ]
            for lkt in range(n_kt):
                r0 = max(0, (lkt - 4 * qb) * 128)
                ks = slice(lkt * 128, (lkt + 1) * 128)
                qs = slice(qb * 512 + r0, (qb + 1) * 512)
                ps2 = pss.tile([128, 1024], f32, tag="s2")
                nc.tensor.matmul(
                    ps2[:, r0:512], kT[hp][0:64, ks], qT[hp][0:64, qs],
                    start=True, stop=True,
                )
                nc.tensor.matmul(
                    ps2[:, 512 + r0 : 1024], kT[hp][64:128, ks], qT[hp][64:128, qs],
                    start=True, stop=True,
                )
                es = esp.tile([128, 1024], bf16, tag="es")
                if r0:
                    nc.scalar.activation(
                        es[:, :].rearrange("p (h q) -> p h q", h=2)[:, :, r0:512],
                        ps2[:, :].rearrange("p (h q) -> p h q", h=2)[:, :, r0:512],
                        AF.Exp,
                        scale=scale,
                    )
                else:
                    nc.scalar.activation(es, ps2, AF.Exp, scale=scale)
                if lkt >= 4 * qb:  # diagonal tile: causal mask, both heads
                    nc.gpsimd.tensor_mul(
                        es[:, :].rearrange("p (h q) -> p h q", h=2)[
                            :, :, r0 : r0 + 128
                        ],
                        es[:, :].rearrange("p (h q) -> p h q", h=2)[
                            :, :, r0 : r0 + 128
                        ],
                        tri2[:, :, :],
                    )
                if feeds:
                    feeds.pop(0)()
                else:
                    emit_keepalive()
                if len(pend) >= 3:
                    issue_pend()
                pend.append(
                    (
                        [
                            dict(
                                out=po[h][0:65, r0:512],
                                lhsT=Vp[hp][:, lkt, h * 65 : (h + 1) * 65],
                                rhs=es[:, h * 512 + r0 : (h + 1) * 512],
                                start=(lkt == 0),
                                stop=(lkt == n_kt - 1),
                            )
                            for h in range(2)
                        ],
                        make_evac(po, hp, qb) if lkt == n_kt - 1 else None,
                    )
                )

        # Schedule: prologue computes t0's qA/kA chains, then the continuous
        # stream starts in pair-interleaved ascending order; everything else
        # flows through the feed queue.
        emit_qkv_group(0, 0)
        emit_qkv_group(0, 1)
        qkv_feed_plan = [
            (0, 2), (0, 3), (0, 4), (0, 5),
            (1, 0), (1, 1), (1, 2), ("dma", 2),
            (1, 3), (1, 4), (1, 5),
            (2, 0), (2, 1), (2, 2), ("dma", 3),
            (2, 3), (2, 4), (2, 5),
            (3, 0), (3, 1), (3, 2),
            (3, 3), (3, 4), (3, 5),
        ]
        for item in qkv_feed_plan:
            if item[0] == "dma":
                feeds.append(lambda tt=item[1]: emit_x_dma(tt))
            else:
                feeds.append(lambda tt=item[0], g=item[1]: emit_qkv_group(tt, g))

        for qb in range(NT):
            emit_att_block(0, qb)
            emit_att_block(1, qb)
        while pend:
            emit_keepalive()
            issue_pend()
        while feeds:
            # keep the HAM clock gate warm through the (serial) tail
            emit_keepalive()
            feeds.pop(0)()
        emit_keepalive()

    nc.finalize()
    _CACHE["nc"] = nc
    return nc


def _prep_inputs(x, w_attn, b_attn, w_proj):
    x = np.ascontiguousarray(np.asarray(x, dtype=np.float32))
    w_attn = np.asarray(w_attn, dtype=np.float32)
    b_attn = np.asarray(b_attn, dtype=np.float32)
    w_proj = np.asarray(w_proj, dtype=np.float32)

    # per batch: xT[p, tt, cc, t] = x[b, tt*512+t, cc*128+p]
    xTs = [
        _to_bf16(x[b].reshape(NT, 512, CCH, 128).transpose(3, 0, 2, 1))
        for b in range(B)
    ]
    in_maps = []
    for c in range(NCORE):
        b = c // 4
        hq = (c % 4) * HPC  # first global head on this core
        blocks = []
        bias_cols = []
        for hp in range(2):
            hs = [hq + 2 * hp, hq + 2 * hp + 1]
            for off in (0, C, 2 * C):  # q, k, v
                for h in hs:
                    blocks.append(w_attn[:, off + h * HD : off + (h + 1) * HD])
                bias_cols.append(
                    np.concatenate(
                        [b_attn[off + h * HD : off + (h + 1) * HD] for h in hs]
                    )
                )
        wq_flat = _to_bf16(np.concatenate(blocks, axis=1))  # [C, 768]
        # group-major: wqkv[p, g, cc, c] = wq_flat[cc*128+p, g*128+c]
        wqkv = np.ascontiguousarray(
            wq_flat.reshape(CCH, 128, 6, 128).transpose(1, 2, 0, 3)
        )
        bqkv = np.ascontiguousarray(
            np.stack(bias_cols, axis=1).astype(np.float32)
        )  # [128, 6]
        wp = _to_bf16(
            w_proj[hq * HD : hq * HD + 256, :].reshape(2, 128, C).transpose(1, 0, 2)
        )  # [128, 2, C]
        sel = np.zeros((33, 128), dtype=np.float32)
        sel[0, 0:64] = 1.0
        sel[32, 64:128] = 1.0
        sel = _to_bf16(sel)
        in_maps.append(
            {"xT": xTs[b], "wqkv": wqkv, "bqkv": bqkv, "wp": wp, "sel": sel}
        )
    return in_maps


def _run(x, w_attn, b_attn, w_proj, b_proj, trace=False, tmpdir=None):
    from concourse.bass_utils import run_bass_kernel_spmd

    nc = _build()
    in_maps = _prep_inputs(x, w_attn, b_attn, w_proj)
    res = run_bass_kernel_spmd(
        nc, in_maps, list(range(NCORE)), trace=trace, tmpdir=tmpdir
    )
    # v-bias folded here: att rows sum to 1, so att@(X Wv + 1 bv^T) Wp + bp
    # == att@(X Wv) Wp + (bp + bv @ Wp)
    bp = np.asarray(b_proj, dtype=np.float64) + (
        np.asarray(b_attn, dtype=np.float64)[2 * C :] @ np.asarray(w_proj, np.float64)
    )
    outs = []
    for b in range(B):
        acc = np.sum(
            np.stack(
                [
                    np.asarray(res.results[b * 4 + i]["out"], dtype=np.float64)
                    for i in range(4)
                ]
            ),
            axis=0,
        )
        outs.append((acc + bp).astype(np.float32))
    return np.stack(outs), res


def kernel(x, w_attn, b_attn, w_proj, b_proj):
    out, _ = _run(x, w_attn, b_attn, w_proj, b_proj, trace=False)
    return out


# revision 62
# speedup vs baseline: 1.0267x; 1.0267x over previous
"""Causal self-attention (B=2, T=2048, C=1024, H=16) on 8 trn2 NeuronCores.

Sharding: batch x head-group. Core c handles batch c//4 and the 4 heads
[4*(c%4), 4*(c%4)+4), as two head-pairs A=(h0,h1), B=(h2,h3). Each core
reads only its batch's half of x (4MB bf16) and writes an 8MB partial
output; the host sums 4 partials per batch and adds b_proj.

Per core:
  - QKV projection of its batch (6 groups of 128 weight cols: qA kA vA
    qB kB vB), producing qT/kT in [head_dim, T] layout and V' in
    [T, head_dim] layout via PE transposes, with a ones column per head
    (softmax denominator accumulates during att@V).
  - Flash-style causal attention per (head-pair, q-block) as ONE
    continuous software-pipelined stream across all 8 blocks in
    pair-interleaved order (A,0),(B,0),(A,1),(B,1),...: concurrent
    row-group S^T matmul pairs into a 2-bank PSUM tile, one ACT exp for
    both heads, triangular 0/1 mask on diagonal tiles (GpSimd), att@V
    pipelined two k-tiles behind S/exp with the att@V pend queue
    surviving block boundaries (no pipeline drain between blocks).
  - A FIFO feed queue interleaves one filler work item per k-tile into
    the PE stream: remaining QKV group chains, x-tile DMAs, softmax
    normalizations (selector-matmul denominator broadcast + fast approx
    reciprocal + in-place multiply) and output-projection chunks. When
    the queue is empty a tiny junk matmul keeps the PE HAM clock gate
    at 8/8 through ACT(exp)-bound stretches.
  - Output projection contracts 256 y-dims (both head-pairs) per
    128-query chunk; partial [2048, 1024] written to DRAM.

Matmuls run in bfloat16 (same PE column throughput as f32r, but half
the DMA/SBUF traffic and lower power -> less HAM/P0 throttling), with
full fp32 PSUM accumulate. rel err ~3e-3 vs the 2e-2 gate.
"""

import sys

sys.path.insert(0, "/opt/trn_rl_repo")

import numpy as np

B, T, C, H, HD = 2, 2048, 1024, 16, 64
NCORE = 8
HPC = 4           # heads per core
NT = T // 512     # 4 T-tiles (one batch per core)
CCH = C // 128    # 8 contraction chunks
NKT = T // 128    # 16 k-tiles

JUNK_N = 160      # keepalive junk matmul width (columns)


def _to_bf16(x):
    import ml_dtypes

    return np.ascontiguousarray(np.asarray(x, dtype=np.float32)).astype(
        ml_dtypes.bfloat16
    )


_CACHE = {}


def _build():
    if "nc" in _CACHE:
        return _CACHE["nc"]
    from contextlib import ExitStack

    import concourse.bass as bass
    import concourse.bacc as bacc
    import concourse.mybir as mybir
    import concourse.tile as tile
    from concourse.masks import make_identity, make_upper_triangular

    f32, bf16 = mybir.dt.float32, mybir.dt.bfloat16
    AF = mybir.ActivationFunctionType

    nc = bacc.Bacc(None, target_bir_lowering=False, debug=False)
    # x pre-permuted on host to [p, tt, cc, t] so each T-tile DMA reads
    # contiguous runs per partition
    xT_d = nc.dram_tensor("xT", [128, NT, CCH, 512], bf16, kind="ExternalInput")
    # weights group-major so the first QKV chain can start after 1 group
    wqkv_d = nc.dram_tensor("wqkv", [128, 6, CCH, 128], bf16, kind="ExternalInput")
    bqkv_d = nc.dram_tensor("bqkv", [128, 6], f32, kind="ExternalInput")
    wp_d = nc.dram_tensor("wp", [128, 2, C], bf16, kind="ExternalInput")
    sel_d = nc.dram_tensor("sel", [33, 128], bf16, kind="ExternalInput")
    # bf16 partials: host sums the 4 per-batch partials in float64
    out_d = nc.dram_tensor("out", [T, C], bf16, kind="ExternalOutput")

    with tile.TileContext(nc) as tc, ExitStack() as ctx:
        sb = ctx.enter_context(tc.tile_pool(name="sb", bufs=1))
        xp = ctx.enter_context(tc.tile_pool(name="xp", bufs=4))
        esp = ctx.enter_context(tc.tile_pool(name="esp", bufs=4))
        rcpp = ctx.enter_context(tc.tile_pool(name="rcpp", bufs=2))
        outp = ctx.enter_context(tc.tile_pool(name="outp", bufs=3))
        # PSUM: 2 + 4 + 2 = 8 banks
        pa = ctx.enter_context(tc.tile_pool(name="pa", bufs=2, space="PSUM"))
        pss = ctx.enter_context(tc.tile_pool(name="pss", bufs=2, space="PSUM"))
        pso = ctx.enter_context(tc.tile_pool(name="pso", bufs=2, space="PSUM"))

        wq_sb = sb.tile([128, 6, CCH, 128], bf16, tag="wq")

        qT = [sb.tile([128, T], bf16, tag=f"qT{hp}", name=f"qT{hp}") for hp in range(2)]
        kT = [sb.tile([128, T], bf16, tag=f"kT{hp}", name=f"kT{hp}") for hp in range(2)]
        # V' per head-pair: per k-tile [V_h0 (64) | 1 | V_h1 (64) | 1] = 130;
        # the ones column accumulates the softmax denominator during att@v
        VW = 130
        Vp = [
            sb.tile([128, NKT, VW], bf16, tag=f"Vp{hp}", name=f"Vp{hp}")
            for hp in range(2)
        ]
        for hp in range(2):
            nc.vector.memset(Vp[hp][:, :, :], 1.0)
        # normalized attention outputs, persistent across all blocks
        ytb = [
            [
                sb.tile([128, 512], bf16, tag=f"ytb{hp}{qb}", name=f"ytb{hp}{qb}")
                for qb in range(NT)
            ]
            for hp in range(2)
        ]
        # selector [33, 128]: col j reads partition 0 (head0 denom) for j<64,
        # partition 32 (head1 denom) for j>=64 -- one K=33 matmul broadcasts
        # both heads' denominator rows into a single [128, 512] PSUM bank
        # (32-aligned partition bases are an ISA requirement)
        sel_sb = sb.tile([33, 128], bf16, tag="sel")
        nc.sync.dma_start(out=sel_sb, in_=sel_d[:, :])
        # one persistent denominator tile per block (norms run lazily many
        # slots after evacuation); rows 1..31/33.. are never written and must
        # be finite (the selector matmul touches all 33 partitions), so
        # memset the whole tiles once
        dh_tiles = {
            (hp, qb): sb.tile([33, 512], bf16, tag=f"dh{hp}{qb}", name=f"dh{hp}{qb}")
            for hp in range(2)
            for qb in range(NT)
        }
        for t in dh_tiles.values():
            nc.vector.memset(t[:, :], 1.0)

        # ---------------- QKV projection ----------------
        x_tiles = [None] * NT

        def emit_x_dma(tt):
            x_t = xp.tile([128, CCH, 512], bf16, tag="x", name=f"x{tt}")
            if tt == 0:
                # chunked loads so the first matmul chain starts early
                nc.sync.dma_start(out=wq_sb[:, 0], in_=wqkv_d[:, 0])
                for cc in range(CCH):
                    nc.sync.dma_start(out=x_t[:, cc, :], in_=xT_d[:, 0, cc, :])
                for g in range(1, 6):
                    nc.sync.dma_start(out=wq_sb[:, g], in_=wqkv_d[:, g])
            else:
                nc.sync.dma_start(out=x_t, in_=xT_d[:, tt, :, :])
            x_tiles[tt] = x_t

        def emit_qkv_group(tt, g):
            # g: 0=qA 1=kA 2=vA 3=qB 4=kB 5=vB
            hp, kind = divmod(g, 3)
            x_t = x_tiles[tt]
            tloc = tt * 512
            if kind == 2:
                ps = pa.tile([128, 4, 128], f32, tag="mm")
                for j in range(4):
                    for cc in range(CCH):
                        nc.tensor.matmul(
                            ps[:, j, :],
                            x_t[:, cc, j * 128 : (j + 1) * 128],
                            wq_sb[:, g, cc, :],
                            start=(cc == 0),
                            stop=(cc == CCH - 1),
                        )
            else:
                ps = pa.tile([128, 512], f32, tag="mm")
                for cc in range(CCH):
                    nc.tensor.matmul(
                        ps,
                        wq_sb[:, g, cc, :],
                        x_t[:, cc, :],
                        start=(cc == 0),
                        stop=(cc == CCH - 1),
                    )
            if kind == 0:
                nc.vector.tensor_scalar_add(
                    qT[hp][:, tloc : tloc + 512], ps, bias_sb[:, g : g + 1]
                )
            elif kind == 1:
                nc.vector.tensor_scalar_add(
                    kT[hp][:, tloc : tloc + 512], ps, bias_sb[:, g : g + 1]
                )
            else:
                # v is produced directly in [k-pos, v-dim] layout by swapping
                # matmul operands (x chunk stationary, w moving): no PE
                # transposes, no v_t staging on DVE. The v bias is folded
                # into b_proj on the host (softmax rows sum to 1).
                for j in range(4):
                    ktl = tt * 4 + j
                    # one strided copy moves both heads' V columns
                    nc.vector.tensor_copy(
                        Vp[hp][:, ktl, 0:130].rearrange("p (s e) -> p s e", s=2)[
                            :, :, 0:64
                        ],
                        ps[:, j, :].rearrange("p (s e) -> p s e", s=2),
                    )

        emit_x_dma(0)
        # constants not needed immediately: emit loads after the x chunks
        bias_sb = sb.tile([128, 6], f32, tag="bias")
        nc.sync.dma_start(out=bias_sb, in_=bqkv_d[:, :])
        emit_x_dma(1)
        wp_sb = sb.tile([128, 2, C], bf16, tag="wp")
        nc.sync.dma_start(out=wp_sb, in_=wp_d[:, :, :])
        ident = sb.tile([128, 128], bf16, tag="ident")
        make_identity(nc, ident)
        tri2 = sb.tile([128, 2, 128], bf16, tag="tri2")
        make_upper_triangular(nc, tri2[:, 0, :], val=1.0, diag=True)
        nc.gpsimd.tensor_copy(tri2[:, 1, :], tri2[:, 0, :])
        # PE warm-up: junk matmuls during the initial DMA wait pull the HAM
        # clock gate to 8/8 before the first real qkv chain issues. The
        # activity window is 4096 cycles (~3.4us) and must be filled
        # continuously, so emit ~4.5us worth of cold-rate matmuls.
        wu = pa.tile([128, 128], f32, tag="mm", name="warmup")
        for _ in range(42):
            nc.tensor.matmul(wu, ident, ident, start=True, stop=True)
        # preload the exp activation table set (~2.7us) off the critical path
        etp = sb.tile([1, 16], f32, tag="etp")
        nc.vector.memset(etp[:, :], 0.0)
        nc.scalar.activation(etp, etp, AF.Exp)

        # ------------- attention: one continuous pipelined stream -------------
        scale = 1.0 / 8.0  # 1/sqrt(HD)
        feeds = []        # FIFO of filler closures, one consumed per k-tile
        pend = []         # att@v groups pipelined two k-tiles behind S/exp

        def issue_pend():
            grp, on_done = pend.pop(0)
            for mm in grp:
                nc.tensor.matmul(**mm)
            if on_done is not None:
                on_done()

        def emit_keepalive():
            # throwaway matmul into a fresh pa tile: keeps the PE HAM
            # activity window busy through ACT(exp)-bound stretches so the
            # real att matmuls stay at the 2.4GHz clock
            return

        def make_norm(dh, hp, qb):
            def norm():
                # broadcast both heads' denominators into one PSUM bank via a
                # single K=33 matmul (selector stationary), one fast approx
                # reciprocal, then normalize ytb in place.
                den = pa.tile([128, 512], f32, tag="mm", name="den")
                nc.tensor.matmul(
                    den, sel_sb[0:33, :], dh[0:33, :], start=True, stop=True
                )
                rcp = rcpp.tile([128, 512], f32, tag="rcp")
                nc.vector.reciprocal_approx_fast(out=rcp, in_=den)
                yt = ytb[hp][qb]
                nc.vector.tensor_mul(yt[:, :], yt[:, :], rcp[:, :])
            return norm

        def make_proj(qb, j):
            def proj():
                out_t = outp.tile([128, C], bf16, tag="out", name="out_t")
                js = slice(j * 128, (j + 1) * 128)
                for ncol in range(2):
                    cs = slice(ncol * 512, (ncol + 1) * 512)
                    pp = pa.tile([128, 512], f32, tag="mm", name="pp")
                    nc.tensor.matmul(
                        pp, ytb[0][qb][:, js], wp_sb[:, 0, cs], start=True, stop=False
                    )
                    nc.tensor.matmul(
                        pp, ytb[1][qb][:, js], wp_sb[:, 1, cs], start=False, stop=True
                    )
                    nc.vector.tensor_copy(out_t[:, cs], pp)
                row = qb * 512 + j * 128
                nc.sync.dma_start(out=out_d[row : row + 128, :], in_=out_t)
            return proj

        def make_evac(po, hp, qb):
            def evac():
                # evacuate po fast so the next block's att@v gets its PSUM
                # banks: denominator rows -> dh partitions 0/32, un-normalized
                # y -> ytb; then queue the (lazy) normalization and, once both
                # head-pairs of qb are evacuated, the output projection.
                dh = dh_tiles[(hp, qb)]
                yt = ytb[hp][qb]
                nc.vector.tensor_copy(dh[0:1, :], po[0][64:65, :])
                nc.vector.tensor_copy(yt[0:64, :], po[0][0:64, :])
                nc.vector.tensor_copy(dh[32:33, :], po[1][64:65, :])
                nc.vector.tensor_copy(yt[64:128, :], po[1][0:64, :])
                feeds.append(make_norm(dh, hp, qb))
                if hp == 1:
                    for j in range(4):
                        feeds.append(make_proj(qb, j))
            return evac

        def emit_att_block(hp, qb):
            n_kt = 4 * (qb + 1)
            po = [
                pso.tile([128, 512], f32, tag="po", name=f"po{hp}{qb}{h}")
                for h in range(2)
            ]
            for lkt in range(n_kt):
                r0 = max(0, (lkt - 4 * qb) * 128)
                ks = slice(lkt * 128, (lkt + 1) * 128)
                qs = slice(qb * 512 + r0, (qb + 1) * 512)
                ps2 = pss.tile([128, 1024], f32, tag="s2")
                nc.tensor.matmul(
                    ps2[:, r0:512], kT[hp][0:64, ks], qT[hp][0:64, qs],
                    start=True, stop=True,
                )
                nc.tensor.matmul(
                    ps2[:, 512 + r0 : 1024], kT[hp][64:128, ks], qT[hp][64:128, qs],
                    start=True, stop=True,
                )
                es = esp.tile([128, 1024], bf16, tag="es")
                if r0:
                    nc.scalar.activation(
                        es[:, :].rearrange("p (h q) -> p h q", h=2)[:, :, r0:512],
                        ps2[:, :].rearrange("p (h q) -> p h q", h=2)[:, :, r0:512],
                        AF.Exp,
                        scale=scale,
                    )
                else:
                    nc.scalar.activation(es, ps2, AF.Exp, scale=scale)
                if lkt >= 4 * qb:  # diagonal tile: causal mask, both heads
                    nc.gpsimd.tensor_mul(
                        es[:, :].rearrange("p (h q) -> p h q", h=2)[
                            :, :, r0 : r0 + 128
                        ],
                        es[:, :].rearrange("p (h q) -> p h q", h=2)[
                            :, :, r0 : r0 + 128
                        ],
                        tri2[:, :, :],
                    )
                if feeds:
                    feeds.pop(0)()
                else:
                    emit_keepalive()
                if len(pend) >= 2:
                    issue_pend()
                pend.append(
                    (
                        [
                            dict(
                                out=po[h][0:65, r0:512],
                                lhsT=Vp[hp][:, lkt, h * 65 : (h + 1) * 65],
                                rhs=es[:, h * 512 + r0 : (h + 1) * 512],
                                start=(lkt == 0),
                                stop=(lkt == n_kt - 1),
                            )
                            for h in range(2)
                        ],
                        make_evac(po, hp, qb) if lkt == n_kt - 1 else None,
                    )
                )

        # Schedule: prologue computes t0's qA/kA chains, then the continuous
        # stream starts in pair-interleaved ascending order; everything else
        # flows through the feed queue.
        emit_qkv_group(0, 0)
        emit_qkv_group(0, 1)
        qkv_feed_plan = [
            (0, 2), (0, 3), (0, 4), (0, 5),
            (1, 0), (1, 1), (1, 2), ("dma", 2),
            (1, 3), (1, 4), (1, 5),
            (2, 0), (2, 1), (2, 2), ("dma", 3),
            (2, 3), (2, 4), (2, 5),
            (3, 0), (3, 1), (3, 2),
            (3, 3), (3, 4), (3, 5),
        ]
        for item in qkv_feed_plan:
            if item[0] == "dma":
                feeds.append(lambda tt=item[1]: emit_x_dma(tt))
            else:
                feeds.append(lambda tt=item[0], g=item[1]: emit_qkv_group(tt, g))

        for qb in range(NT):
            emit_att_block(0, qb)
            emit_att_block(1, qb)
        while pend:
            emit_keepalive()
            issue_pend()
        while feeds:
            # keep the HAM clock gate warm through the (serial) tail
            emit_keepalive()
            feeds.pop(0)()
        emit_keepalive()

    nc.finalize()
    _CACHE["nc"] = nc
    return nc


def _prep_inputs(x, w_attn, b_attn, w_proj):
    x = np.ascontiguousarray(np.asarray(x, dtype=np.float32))
    w_attn = np.asarray(w_attn, dtype=np.float32)
    b_attn = np.asarray(b_attn, dtype=np.float32)
    w_proj = np.asarray(w_proj, dtype=np.float32)

    # per batch: xT[p, tt, cc, t] = x[b, tt*512+t, cc*128+p]
    xTs = [
        _to_bf16(x[b].reshape(NT, 512, CCH, 128).transpose(3, 0, 2, 1))
        for b in range(B)
    ]
    in_maps = []
    for c in range(NCORE):
        b = c // 4
        hq = (c % 4) * HPC  # first global head on this core
        blocks = []
        bias_cols = []
        for hp in range(2):
            hs = [hq + 2 * hp, hq + 2 * hp + 1]
            for off in (0, C, 2 * C):  # q, k, v
                for h in hs:
                    blocks.append(w_attn[:, off + h * HD : off + (h + 1) * HD])
                bias_cols.append(
                    np.concatenate(
                        [b_attn[off + h * HD : off + (h + 1) * HD] for h in hs]
                    )
                )
        wq_flat = _to_bf16(np.concatenate(blocks, axis=1))  # [C, 768]
        # group-major: wqkv[p, g, cc, c] = wq_flat[cc*128+p, g*128+c]
        wqkv = np.ascontiguousarray(
            wq_flat.reshape(CCH, 128, 6, 128).transpose(1, 2, 0, 3)
        )
        bqkv = np.ascontiguousarray(
            np.stack(bias_cols, axis=1).astype(np.float32)
        )  # [128, 6]
        wp = _to_bf16(
            w_proj[hq * HD : hq * HD + 256, :].reshape(2, 128, C).transpose(1, 0, 2)
        )  # [128, 2, C]
        sel = np.zeros((33, 128), dtype=np.float32)
        sel[0, 0:64] = 1.0
        sel[32, 64:128] = 1.0
        sel = _to_bf16(sel)
        in_maps.append(
            {"xT": xTs[b], "wqkv": wqkv, "bqkv": bqkv, "wp": wp, "sel": sel}
        )
    return in_maps


def _run(x, w_attn, b_attn, w_proj, b_proj, trace=False, tmpdir=None):
    from concourse.bass_utils import run_bass_kernel_spmd

    nc = _build()
    in_maps = _prep_inputs(x, w_attn, b_attn, w_proj)
    res = run_bass_kernel_spmd(
        nc, in_maps, list(range(NCORE)), trace=trace, tmpdir=tmpdir
    )
    # v-bias folded here: att rows sum to 1, so att@(X Wv + 1 bv^T) Wp + bp
    # == att@(X Wv) Wp + (bp + bv @ Wp)
    bp = np.asarray(b_proj, dtype=np.float64) + (
        np.asarray(b_attn, dtype=np.float64)[2 * C :] @ np.asarray(w_proj, np.float64)
    )
    outs = []
    for b in range(B):
        acc = np.sum(
            np.stack(
                [
                    np.asarray(res.results[b * 4 + i]["out"], dtype=np.float64)
                    for i in range(4)
                ]
            ),
            axis=0,
        )
        outs.append((acc + bp).astype(np.float32))
    return np.stack(outs), res


def kernel(x, w_attn, b_attn, w_proj, b_proj):
    out, _ = _run(x, w_attn, b_attn, w_proj, b_proj, trace=False)
    return out
